# revision 9
# baseline (speedup 1.0000x reference)
"""BA3Net (3-layer LEConv GNN + BN + mean-pool + MLP head) on 8 TRN2 NeuronCores.

LEConv layer algebra: with A_w[i,j] = sum of EdgeAttr over edges j->i and
deg[i] = weighted in-degree:

    agg = segsum(w*(a[src]-b[dst]), dst)       where a = h@W1+b1, b = h@W2
        = (A_w h) @ W1 + deg*b1 - (deg . h) @ W2
    h'  = agg + h@W3 + b3

So the only sparse op per layer is p = A_w @ h (same matrix every layer).

Distribution: dst-nodes sharded 8 ways (12544 padded/core). Per layer each
core computes h'^T for its shard, BN stats are AllReduced, and the node-major
bf16 h-table (gather source, 256B rows) is AllGathered into shared DRAM.
The SpMM gathers h[src] rows with gpsimd.dma_gather on 4 SWDGE queues
(~2.5ns/edge, descriptor-generation bound) and aggregates on TensorE with
weighted indicator matmuls; indicators are built on VectorE by iota-compare.
Edges are sorted by (PSUM-pass, src-chunk, dst-block) with per-bin capacities
equalized across cores so the instruction stream is SPMD-uniform.
"""
import sys

sys.path.insert(0, "/opt/trn_rl_repo")

import numpy as np
import ml_dtypes

import concourse.bass as bass
import concourse.bacc as bacc
import concourse.mybir as mybir
import concourse.tile as tile
from concourse.bass_utils import run_bass_kernel_spmd
from concourse.library_config import mlp as mlp_lib

BF16 = mybir.dt.bfloat16
F32 = mybir.dt.float32
I16 = mybir.dt.int16

NC = 8
N = 100000
E = 1200000
D = 64
G = 256
L = 3
SHR = 12500           # real nodes per shard
SH = 12544            # padded nodes per shard (= 98 blocks of 128)
NBLK = SH // 128      # 98
NP = NC * SH          # 100352
CHUNK = NP // 4       # 25088 (< 2^15 so gather idx fits int16)
BPP = 4               # dst blocks per PSUM pass (1 PSUM bank each)
NPASS = (NBLK + BPP - 1) // BPP   # 9
TB = 8                # tiles per indicator-build batch
MAXCALL = 12544       # max idxs per dma_gather call (multiple of 128)
BN_EPS = 1e-5


def _pass_blocks(p):
    return range(p * BPP, min(NBLK, (p + 1) * BPP))


# ----------------------------------------------------------------------------
# Host preprocessing
# ----------------------------------------------------------------------------

def preprocess(x, EdgeID, EdgeAttr, batch):
    src = np.asarray(EdgeID[0], dtype=np.int64)
    dst = np.asarray(EdgeID[1], dtype=np.int64)
    w = np.asarray(EdgeAttr, dtype=np.float32)
    batch = np.asarray(batch, dtype=np.int64)
    x = np.asarray(x, dtype=np.float32)

    core = dst // SHR
    dl = (dst - core * SHR).astype(np.int64)
    blk = dl // 128
    pas = blk // BPP
    src_p = (src // SHR) * SH + (src % SHR)
    chunk = src_p // CHUNK
    sloc = (src_p - chunk * CHUNK).astype(np.int16)

    counts = np.zeros((NC, 4, NBLK), dtype=np.int64)
    np.add.at(counts, (core, chunk, blk), 1)
    caps = np.maximum(counts.max(axis=0), 128)          # [4, 98]

    bin_start = np.zeros((4, NBLK), dtype=np.int64)
    run_start = {}
    run_cap = {}
    pos = 0
    for p in range(NPASS):
        for k in range(4):
            run_start[(p, k)] = pos
            for b in _pass_blocks(p):
                bin_start[k, b] = pos
                pos += int(caps[k, b])
            tot = pos - run_start[(p, k)]
            pad = (-tot) % 128
            pos += pad
            run_cap[(p, k)] = tot + pad
    EP = pos
    TP = EP // 128

    b_pri = np.zeros(TP, dtype=np.int64)
    has2 = np.zeros(TP, dtype=bool)
    for p in range(NPASS):
        blocks = list(_pass_blocks(p))
        for k in range(4):
            rs, rc = run_start[(p, k)], run_cap[(p, k)]
            ends = np.cumsum([caps[k, b] for b in blocks])
            for trel in range(rc // 128):
                t = rs // 128 + trel
                p0 = trel * 128
                j = min(int(np.searchsorted(ends, p0, side="right")),
                        len(blocks) - 1)
                b_pri[t] = blocks[j]
                if j + 1 < len(blocks) and ends[j] < p0 + 128:
                    has2[t] = True

    # first/last matmul index per dst block, over run-major emission
    first, last = {}, {}
    i = 0
    for p in range(NPASS):
        for k in range(4):
            rs, rc = run_start[(p, k)], run_cap[(p, k)]
            for trel in range(rc // 128):
                t = rs // 128 + trel
                for b in ([int(b_pri[t])] +
                          ([int(b_pri[t]) + 1] if has2[t] else [])):
                    if b not in first:
                        first[b] = i
                    last[b] = i
                    i += 1

    # position assignment per core
    order_key = np.lexsort((blk, chunk, pas, core))
    ck = (core * 4 + chunk) * NBLK + blk
    ck_sorted = ck[order_key]
    grp_change = np.r_[True, ck_sorted[1:] != ck_sorted[:-1]]
    grp_first = np.where(grp_change)[0]
    grp_id = np.cumsum(grp_change) - 1
    rank = np.arange(E) - grp_first[grp_id]
    bs = bin_start[chunk[order_key], blk[order_key]]
    epos = np.empty(E, dtype=np.int64)
    epos[order_key] = bs + rank

    idx16 = np.zeros((NC, EP), dtype=np.int16)
    slot_a = np.full((NC, EP), -1000.0, dtype=np.float32)
    w_a = np.zeros((NC, EP), dtype=np.float32)
    slot_val = dl - 128 * b_pri[epos // 128]
    assert slot_val.min() >= 0 and slot_val.max() < 256
    idx16[core, epos] = sloc
    slot_a[core, epos] = slot_val
    w_a[core, epos] = w

    idx_l = idx16.reshape(NC, EP // 16, 16).transpose(0, 2, 1)
    idx_rep = np.tile(idx_l, (1, 8, 1)).astype(np.int16)          # [NC,128,EP/16]
    slot_t = slot_a.reshape(NC, TP, 128).transpose(0, 2, 1).astype(np.float32)
    w_t = w_a.reshape(NC, TP, 128).transpose(0, 2, 1).astype(np.float32)

    deg = np.zeros(N, dtype=np.float64)
    np.add.at(deg, dst, w.astype(np.float64))
    deg_sh = np.zeros((NC, 1, SH), dtype=np.float32)
    deg_sh[:, 0, :SHR] = deg.astype(np.float32).reshape(NC, SHR)
    deg_rep = np.repeat(deg_sh, D, axis=1).astype(ml_dtypes.bfloat16)
    degone = np.zeros((NC, 2, SH), dtype=np.float32)
    degone[:, 0, :] = deg_sh[:, 0, :]
    degone[:, 1, :SHR] = 1.0
    degone = degone.astype(ml_dtypes.bfloat16)

    xT = np.zeros((NC, 5, SH), dtype=np.float32)
    xT[:, 0:4, :SHR] = x.reshape(NC, SHR, 4).transpose(0, 2, 1)
    xT[:, 4, :SHR] = 1.0
    xTb = xT.astype(ml_dtypes.bfloat16)

    bslot = np.full((NC, SH), -1000.0, dtype=np.float32)
    bslot[:, :SHR] = batch.reshape(NC, SHR)
    bslot_t = bslot.reshape(NC, NBLK, 128).transpose(0, 2, 1).astype(np.float32)

    meta = dict(EP=EP, TP=TP, run_start=run_start, run_cap=run_cap,
                b_pri=b_pri, has2=has2, first=first, last=last,
                maxrt=max(rc // 128 for rc in run_cap.values()))
    percore = dict(idxs=idx_rep, slot=slot_t, wt=w_t, degone=degone,
                   deg_rep=deg_rep, xT=xTb, bslot=bslot_t)
    return meta, percore


def prep_weights(W_emb, b_emb, W1, b1, W2, W3, b3, gamma, beta,
                 W_l1, b_l1, W_l2, b_l2):
    bf = ml_dtypes.bfloat16
    f = lambda a: np.asarray(a, np.float32)
    out = {}
    out["Wemb"] = np.concatenate([f(W_emb), f(b_emb)[None, :]], 0).astype(bf)
    out["W1b1"] = np.stack([
        np.concatenate([f(W1[l]), f(b1[l])[None, :], f(b3[l])[None, :]], 0)
        for l in range(L)]).astype(bf)                              # [3,66,64]
    out["W32"] = np.stack([
        np.concatenate([f(W3[l]), -f(W2[l])], 0) for l in range(L)]).astype(bf)
    out["gamT"] = f(gamma).reshape(L, D, 1)
    out["betT"] = f(beta).reshape(L, D, 1)
    out["Wl1"] = np.concatenate([f(W_l1), f(b_l1)[None, :]], 0).astype(bf)
    out["Wl2"] = np.concatenate([f(W_l2), f(b_l2)[None, :]], 0).astype(bf)
    out["identb"] = np.eye(128, dtype=np.float32).astype(bf)
    out["identf"] = np.eye(128, dtype=np.float32)
    out["iota"] = np.tile(np.arange(256, dtype=np.float32)[None, :],
                          (128, 1)).astype(bf)
    return out


# ----------------------------------------------------------------------------
# Device graph
# ----------------------------------------------------------------------------

def build(meta, debug=False):
    EP, TP = meta["EP"], meta["TP"]
    run_start, run_cap = meta["run_start"], meta["run_cap"]
    b_pri, has2 = meta["b_pri"], meta["has2"]
    first, last = meta["first"], meta["last"]
    MAXRT = meta["maxrt"]

    nc = bacc.Bacc("TRN2", debug=False, num_swdge_queues=4)

    d_idxs = nc.dram_tensor("idxs", [128, EP // 16], I16, kind="ExternalInput")
    d_slot = nc.dram_tensor("slot", [128, TP], F32, kind="ExternalInput")
    d_wt = nc.dram_tensor("wt", [128, TP], F32, kind="ExternalInput")
    d_deg = nc.dram_tensor("degone", [2, SH], BF16, kind="ExternalInput")
    d_degr = nc.dram_tensor("deg_rep", [D, SH], BF16, kind="ExternalInput")
    d_xT = nc.dram_tensor("xT", [5, SH], BF16, kind="ExternalInput")
    d_bslot = nc.dram_tensor("bslot", [128, NBLK], F32, kind="ExternalInput")
    d_Wemb = nc.dram_tensor("Wemb", [5, D], BF16, kind="ExternalInput")
    d_W1b1 = nc.dram_tensor("W1b1", [L, 66, D], BF16, kind="ExternalInput")
    d_W32 = nc.dram_tensor("W32", [L, 128, D], BF16, kind="ExternalInput")
    d_gamT = nc.dram_tensor("gamT", [L, D, 1], F32, kind="ExternalInput")
    d_betT = nc.dram_tensor("betT", [L, D, 1], F32, kind="ExternalInput")
    d_Wl1 = nc.dram_tensor("Wl1", [65, D], BF16, kind="ExternalInput")
    d_Wl2 = nc.dram_tensor("Wl2", [65, 3], BF16, kind="ExternalInput")
    d_identb = nc.dram_tensor("identb", [128, 128], BF16, kind="ExternalInput")
    d_identf = nc.dram_tensor("identf", [128, 128], F32, kind="ExternalInput")
    d_iota = nc.dram_tensor("iota", [128, 256], BF16, kind="ExternalInput")
    d_out = nc.dram_tensor("out", [G, 3], F32, kind="ExternalOutput")
    d_tbls = [nc.dram_tensor(f"tbl{i}", [NP, 128], BF16, addr_space="Shared")
              for i in range(L)]
    d_dbg = (nc.dram_tensor("dbg", [5, 128, SH], BF16, kind="ExternalOutput")
             if debug else None)
    d_tdump = (nc.dram_tensor("tdump", [NP, 128], BF16, kind="ExternalOutput")
               if debug else None)
    d_gdump = (nc.dram_tensor("gdump", [128, meta["maxrt"], 128], BF16,
                              kind="ExternalOutput") if debug else None)

    AluOp = mybir.AluOpType
    AF = mybir.ActivationFunctionType

    with tile.TileContext(nc) as tc:
        with (
            tc.tile_pool(name="const", bufs=1) as cp,
            tc.tile_pool(name="big", bufs=1) as bigp,
            tc.tile_pool(name="gath", bufs=8) as gp,
            tc.tile_pool(name="ind", bufs=3) as ip,
            tc.tile_pool(name="ixp", bufs=8) as ixp,
            tc.tile_pool(name="work", bufs=1) as wp,
            tc.tile_pool(name="dram", bufs=1, space="DRAM") as dp,
            tc.tile_pool(name="dram2", bufs=2, space="DRAM") as dp2,
        ):
            from contextlib import ExitStack
            ps_stack = ExitStack()
            psP = ps_stack.enter_context(
                tc.tile_pool(name="psP", bufs=4, space="PSUM"))
            psE = ps_stack.enter_context(
                tc.tile_pool(name="psE", bufs=2, space="PSUM"))
            psT = ps_stack.enter_context(
                tc.tile_pool(name="psT", bufs=2, space="PSUM"))
            nc.gpsimd.load_library(mlp_lib)

            def load(pool, dten, tag):
                t = pool.tile(list(dten.shape), dten.dtype, tag=tag)
                nc.sync.dma_start(t[:], dten[:])
                return t

            slot = load(cp, d_slot, "slot")
            wt = load(cp, d_wt, "wt")
            degr = load(cp, d_degr, "degr")
            xT = load(cp, d_xT, "xT")
            bslot = load(cp, d_bslot, "bslot")
            Wemb = load(cp, d_Wemb, "Wemb")
            gamT = cp.tile([D, L], F32, tag="gamT")
            nc.sync.dma_start(gamT[:], d_gamT[:].rearrange("l p one -> p (l one)"))
            betT = cp.tile([D, L], F32, tag="betT")
            nc.sync.dma_start(betT[:], d_betT[:].rearrange("l p one -> p (l one)"))
            Wl1 = load(cp, d_Wl1, "Wl1")
            Wl2 = load(cp, d_Wl2, "Wl2")
            identb = load(cp, d_identb, "identb")
            identf = load(cp, d_identf, "identf")
            iota = load(cp, d_iota, "iota")
            W1b1 = cp.tile([66, L, D], BF16, tag="W1b1")
            nc.sync.dma_start(W1b1[:], d_W1b1[:].rearrange("l p f -> p l f"))
            W32 = cp.tile([128, L, D], BF16, tag="W32")
            nc.sync.dma_start(W32[:], d_W32[:].rearrange("l p f -> p l f"))

            hcur = bigp.tile([128, SH], BF16)      # 0:64 hT, 64:128 deg.hT
            hpre = bigp.tile([D, SH], BF16)        # pre-BN h'
            pT = bigp.tile([66, SH], BF16)         # 0:64 p^T, 64 deg, 65 ones
            hnode = bigp.tile([128, NBLK * 128], BF16)

            nc.sync.dma_start(pT[64:66, :], d_deg[:])

            def dense_chunks():
                o = 0
                while o < SH:
                    yield o, min(512, SH - o)
                    o += 512

            def deg_mult():
                nc.vector.tensor_tensor(hcur[D:128, :], hcur[0:D, :], degr[:],
                                        op=AluOp.mult)

            def emb_layer():
                for o, n in dense_chunks():
                    acc = psE.tile([D, 512], F32, tag="acc")
                    nc.tensor.matmul(acc[:, 0:n], Wemb[:], xT[:, o:o + n],
                                     start=True, stop=True)
                    nc.scalar.activation(hcur[0:D, o:o + n], acc[:, 0:n], AF.Copy)
                deg_mult()

            def to_node_major():
                for b in range(NBLK):
                    tp = psT.tile([128, 64], BF16, tag="tp")
                    nc.tensor.transpose(tp[:], hcur[0:D, 128 * b:128 * (b + 1)],
                                        identb[0:D, 0:D])
                    nc.vector.tensor_copy(hnode[:, b * 128:b * 128 + 64], tp[:])

            def allgather_table(i):
                bounce = dp2.tile([SH, 128], BF16, tag="agin")
                tbl = d_tbls[i]
                nc.sync.dma_start(
                    bounce[:].rearrange("(t p) e -> p t e", p=128),
                    hnode[:].rearrange("p (t e) -> p t e", e=128))
                nc.gpsimd.collective_compute(
                    "AllGather", AluOp.bypass,
                    replica_groups=[list(range(NC))],
                    ins=[bounce.opt()], outs=[tbl[:].opt()])
                return tbl

            def spmm(tbl, dumpg=False):
                mmc = 0
                for p in range(NPASS):
                    blocks = list(_pass_blocks(p))
                    paccs = {b: psP.tile([D, 128], F32, tag="pacc",
                                         name=f"pacc{b}")
                             for b in blocks}
                    gts = {}
                    for k in range(4):
                        rs, rc = run_start[(p, k)], run_cap[(p, k)]
                        gt = gp.tile([128, MAXRT, 128], BF16, tag="gt")
                        idxt = ixp.tile([128, MAXRT * 8], I16, tag="idxt")
                        nc.sync.dma_start(
                            idxt[:, 0:rc // 16],
                            d_idxs[:, rs // 16:(rs + rc) // 16])
                        off = 0
                        while off < rc:
                            n = min(MAXCALL, rc - off)
                            nc.gpsimd.dma_gather(
                                gt[:, off // 128:(off + n) // 128, :],
                                tbl[k * CHUNK:(k + 1) * CHUNK, :],
                                idxt[:, off // 16:(off + n) // 16],
                                n, n, 128, single_packet=False, queue_num=k)
                            off += n
                        gts[k] = gt
                    for k in range(4):
                        rs, rc = run_start[(p, k)], run_cap[(p, k)]
                        nt = rc // 128
                        t_base = rs // 128
                        gt = gts[k]
                        if dumpg and p == 0 and k == 0:
                            nc.sync.dma_start(d_gdump[:], gt[:])
                        for t0 in range(0, nt, TB):
                            tbn = min(TB, nt - t0)
                            tg0 = t_base + t0
                            ind = ip.tile([128, TB, 256], BF16, tag="ind")
                            for trel in range(tbn):
                                tg = tg0 + trel
                                nc.vector.tensor_scalar(
                                    ind[:, trel, :], iota[:],
                                    slot[:, tg:tg + 1], wt[:, tg:tg + 1],
                                    op0=AluOp.is_equal, op1=AluOp.mult)
                            for trel in range(tbn):
                                tg = tg0 + trel
                                for half in (0, 1):
                                    if half == 1 and not has2[tg]:
                                        continue
                                    b = int(b_pri[tg]) + half
                                    nc.tensor.matmul(
                                        paccs[b][:],
                                        gt[:, t0 + trel, 0:64],
                                        ind[:, trel, half * 128:half * 128 + 128],
                                        start=(first[b] == mmc),
                                        stop=(last[b] == mmc),
                                        skip_group_check=True)
                                    mmc += 1
                    for b in blocks:
                        nc.scalar.activation(pT[0:D, b * 128:(b + 1) * 128],
                                             paccs[b][:], AF.Copy)

            def epilogue(lidx):
                for o, n in dense_chunks():
                    acc = psE.tile([D, 512], F32, tag="acc")
                    nc.tensor.matmul(acc[:, 0:n], W1b1[:, lidx, :], pT[:, o:o + n],
                                     start=True, stop=False)
                    nc.tensor.matmul(acc[:, 0:n], W32[:, lidx, :], hcur[:, o:o + n],
                                     start=False, stop=True)
                    nc.scalar.activation(hpre[:, o:o + n], acc[:, 0:n], AF.Copy)
                nc.vector.memset(hpre[:, SHR:SH], 0.0)

            def bn_relu(lidx):
                s = wp.tile([D, 2], F32, tag="stats")
                nc.vector.tensor_reduce(s[:, 0:1], hpre[:], mybir.AxisListType.X,
                                        AluOp.add)
                nc.scalar.square(pT[0:D, :], hpre[:])
                nc.vector.tensor_reduce(s[:, 1:2], pT[0:D, :],
                                        mybir.AxisListType.X, AluOp.add)
                bnc = dp.tile([D, 2], F32, tag="bnin")
                bno = dp.tile([D, 2], F32, tag="bnout", addr_space="Shared")
                nc.sync.dma_start(bnc[:], s[:])
                nc.gpsimd.collective_compute(
                    "AllReduce", AluOp.add, replica_groups=[list(range(NC))],
                    ins=[bnc.opt()], outs=[bno.opt()])
                sg = wp.tile([D, 2], F32, tag="statsg")
                nc.sync.dma_start(sg[:], bno[:])
                mean = wp.tile([D, 1], F32, tag="mean")
                var = wp.tile([D, 1], F32, tag="var")
                alph = wp.tile([D, 1], F32, tag="alph")
                bet2 = wp.tile([D, 1], F32, tag="bet2")
                nc.vector.tensor_scalar_mul(mean[:], sg[:, 0:1], 1.0 / N)
                nc.vector.tensor_scalar_mul(var[:], sg[:, 1:2], 1.0 / N)
                nc.vector.tensor_tensor(alph[:], mean[:], mean[:], op=AluOp.mult)
                nc.vector.tensor_tensor(var[:], var[:], alph[:], op=AluOp.subtract)
                nc.vector.tensor_scalar_add(var[:], var[:], BN_EPS)
                nc.scalar.sqrt(var[:], var[:])
                nc.vector.reciprocal(var[:], var[:])
                nc.vector.tensor_tensor(alph[:], gamT[:, lidx:lidx+1], var[:],
                                        op=AluOp.mult)
                nc.vector.tensor_tensor(bet2[:], mean[:], alph[:], op=AluOp.mult)
                nc.vector.tensor_tensor(bet2[:], betT[:, lidx:lidx+1], bet2[:],
                                        op=AluOp.subtract)
                nc.scalar.activation(hcur[0:D, :], hpre[:], AF.Relu,
                                     bias=bet2[:], scale=alph[:])
                nc.vector.memset(hcur[0:D, SHR:SH], 0.0)
                deg_mult()

            # ---------------- main program ----------------
            def dump(i):
                if d_dbg is not None:
                    nc.sync.dma_start(d_dbg[i], hcur[:])
            emb_layer()
            dump(0)
            to_node_major()
            tbl = allgather_table(0)
            if d_tdump is not None:
                nc.sync.dma_start(d_tdump[:], tbl[:])
            for lidx in range(L):
                spmm(tbl, dumpg=(d_gdump is not None and lidx == 0))
                if d_dbg is not None and lidx == 0:
                    nc.sync.dma_start(d_dbg[4, 0:66, :], pT[:])
                epilogue(lidx)
                bn_relu(lidx)
                dump(1 + lidx)
                if lidx < L - 1:
                    to_node_major()
                    tbl = allgather_table(lidx + 1)

            # ---------------- pooling + MLP head ----------------
            to_node_major()
            ps_stack.close()
            psG = ps_stack.enter_context(
                tc.tile_pool(name="psG", bufs=2, space="PSUM"))
            psM = ps_stack.enter_context(
                tc.tile_pool(name="psM", bufs=2, space="PSUM"))
            hv3 = hnode[:].rearrange("p (t e) -> p t e", e=128)
            nc.vector.memset(hv3[:, :, 64:65], 1.0)
            gacc0 = psG.tile([128, 65], F32, tag="gacc")
            gacc1 = psG.tile([128, 65], F32, tag="gacc")
            for t0 in range(0, NBLK, TB):
                tbn = min(TB, NBLK - t0)
                pind = ip.tile([128, TB, 256], BF16, tag="ind")
                for trel in range(tbn):
                    t = t0 + trel
                    nc.vector.tensor_scalar(
                        pind[:, trel, :], iota[:], bslot[:, t:t + 1], None,
                        op0=AluOp.is_equal)
                for trel in range(tbn):
                    t = t0 + trel
                    nc.tensor.matmul(gacc0[:], pind[:, trel, 0:128],
                                     hv3[:, t, 0:65],
                                     start=(t == 0), stop=(t == NBLK - 1),
                                     skip_group_check=True)
                    nc.tensor.matmul(gacc1[:], pind[:, trel, 128:256],
                                     hv3[:, t, 0:65],
                                     start=(t == 0), stop=(t == NBLK - 1),
                                     skip_group_check=True)
            gsb = wp.tile([128, 2, 65], F32, tag="gsb")
            nc.vector.tensor_copy(gsb[:, 0, :], gacc0[:])
            nc.vector.tensor_copy(gsb[:, 1, :], gacc1[:])
            pc_in = dp.tile([128, 2, 65], F32, tag="plin")
            pc_out = dp.tile([128, 2, 65], F32, tag="plout", addr_space="Shared")
            nc.sync.dma_start(pc_in[:], gsb[:])
            nc.gpsimd.collective_compute(
                "AllReduce", AluOp.add, replica_groups=[list(range(NC))],
                ins=[pc_in.opt()], outs=[pc_out.opt()])
            nc.sync.dma_start(gsb[:], pc_out[:])
            gT = wp.tile([65, 256], BF16, tag="gT")
            for wdw in range(2):
                cnt = wp.tile([128, 1], F32, tag="cnt")
                nc.vector.tensor_scalar_max(cnt[:], gsb[:, wdw, 64:65], 1.0)
                nc.vector.reciprocal(cnt[:], cnt[:])
                nc.vector.tensor_scalar(gsb[:, wdw, 0:64], gsb[:, wdw, 0:64],
                                        cnt[:], None, op0=AluOp.mult)
                tp2 = psM.tile([128, 256], F32, tag="mlp")
                nc.tensor.transpose(tp2[0:64, 0:128], gsb[:, wdw, 0:64], identf[:])
                nc.vector.tensor_copy(gT[0:64, wdw * 128:(wdw + 1) * 128], tp2[0:64, 0:128])
            nc.vector.memset(gT[64:65, :], 1.0)
            z = psM.tile([128, 256], F32, tag="mlp")
            nc.tensor.matmul(z[0:64, :], Wl1[:], gT[:], start=True, stop=True)
            zsb = wp.tile([65, 256], BF16, tag="zsb")
            nc.scalar.activation(zsb[0:64, :], z[0:64, :], AF.Relu)
            nc.vector.memset(zsb[64:65, :], 1.0)
            pred = psM.tile([128, 256], F32, tag="mlp")
            nc.tensor.matmul(pred[0:3, :], Wl2[:], zsb[:], start=True, stop=True)
            psb = wp.tile([3, 256], F32, tag="psb")
            nc.vector.tensor_copy(psb[:], pred[0:3, :])
            for hf in range(2):
                tp3 = psM.tile([128, 256], F32, tag="mlp")
                nc.tensor.transpose(tp3[0:128, 0:3], psb[:, hf * 128:(hf + 1) * 128],
                                    identf[0:3, 0:3])
                osb = wp.tile([128, 3], F32, tag="osb")
                nc.vector.tensor_copy(osb[:], tp3[0:128, 0:3])
                nc.sync.dma_start(d_out[hf * 128:(hf + 1) * 128, :], osb[:])
            ps_stack.close()

    nc.compile()
    return nc


# ----------------------------------------------------------------------------
# Entry point
# ----------------------------------------------------------------------------

_cache = {}


def kernel(x, EdgeID, EdgeAttr, batch, W_emb, b_emb, W1, b1, W2, W3, b3,
           gamma, beta, W_l1, b_l1, W_l2, b_l2, _trace=False, _debug=False):
    meta, percore = preprocess(x, EdgeID, EdgeAttr, batch)
    wts = prep_weights(W_emb, b_emb, W1, b1, W2, W3, b3, gamma, beta,
                       W_l1, b_l1, W_l2, b_l2)
    key = ("nc", meta["EP"], bool(_debug))
    if key not in _cache:
        _cache[key] = build(meta, debug=_debug)
    nc = _cache[key]
    in_maps = []
    for c in range(NC):
        m = dict(wts)
        m["idxs"] = percore["idxs"][c]
        m["slot"] = percore["slot"][c]
        m["wt"] = percore["wt"][c]
        m["degone"] = percore["degone"][c]
        m["deg_rep"] = percore["deg_rep"][c]
        m["xT"] = percore["xT"][c]
        m["bslot"] = percore["bslot"][c]
        in_maps.append(m)
    res = run_bass_kernel_spmd(nc, in_maps, list(range(NC)), trace=_trace)
    out = np.asarray(res.results[0]["out"], dtype=np.float32)
    kernel.last_result = res
    if _trace:
        kernel.last_exec_ns = res.exec_time_ns
    if _debug:
        kernel.dbg = [np.asarray(r["dbg"]) for r in res.results]
        kernel.tdump = [np.asarray(r["tdump"]) for r in res.results]
        kernel.gdump = [np.asarray(r["gdump"]) for r in res.results]
    return out



# revision 16
# speedup vs baseline: 2.2784x; 2.2784x over previous
"""BA3Net (3-layer LEConv GNN + BN + mean-pool + MLP head) on 8 TRN2 NeuronCores.

LEConv layer algebra: with A_w[i,j] = sum of EdgeAttr over edges j->i and
deg[i] = weighted in-degree:

    agg = segsum(w*(a[src]-b[dst]), dst)       where a = h@W1+b1, b = h@W2
        = (A_w h) @ W1 + deg*b1 - (deg . h) @ W2
    h'  = agg + h@W3 + b3

So the only sparse op per layer is p = A_w @ h (same matrix every layer).

Distribution: dst-nodes sharded 8 ways (12544 padded/core). Per layer each
core computes h'^T for its shard, BN stats are AllReduced, and the node-major
bf16 h-table (gather source, 256B rows) is AllGathered into shared DRAM.
The SpMM gathers h[src] rows with gpsimd.dma_gather on 4 SWDGE queues
(~2.5ns/edge, descriptor-generation bound) and aggregates on TensorE with
weighted indicator matmuls; indicators are built on VectorE by iota-compare.
Edges are sorted by (PSUM-pass, src-chunk, dst-block) with per-bin capacities
equalized across cores so the instruction stream is SPMD-uniform.
"""
import sys

sys.path.insert(0, "/opt/trn_rl_repo")

import numpy as np
import ml_dtypes

import concourse.bass as bass
import concourse.bacc as bacc
import concourse.mybir as mybir
import concourse.tile as tile
from concourse.bass_utils import run_bass_kernel_spmd
from concourse.library_config import mlp as mlp_lib

BF16 = mybir.dt.bfloat16
F32 = mybir.dt.float32
I16 = mybir.dt.int16

NC = 8
N = 100000
E = 1200000
D = 64
G = 256
L = 3
SHR = 12500           # real nodes per shard
SH = 12544            # padded nodes per shard (= 98 blocks of 128)
NBLK = SH // 128      # 98
NP = NC * SH          # 100352
CHUNK = NP // 4       # 25088 (< 2^15 so gather idx fits int16)
BPP = 4               # dst blocks per PSUM pass (1 PSUM bank each)
NPASS = (NBLK + BPP - 1) // BPP   # 9
TB = 8                # tiles per indicator-build batch
MAXCALL = 12544       # max idxs per dma_gather call (multiple of 128)
BN_EPS = 1e-5


def _pass_blocks(p):
    return range(p * BPP, min(NBLK, (p + 1) * BPP))


# ----------------------------------------------------------------------------
# Host preprocessing
# ----------------------------------------------------------------------------

def preprocess(x, EdgeID, EdgeAttr, batch):
    src = np.asarray(EdgeID[0], dtype=np.int64)
    dst = np.asarray(EdgeID[1], dtype=np.int64)
    w = np.asarray(EdgeAttr, dtype=np.float32)
    batch = np.asarray(batch, dtype=np.int64)
    x = np.asarray(x, dtype=np.float32)

    core = dst // SHR
    dl = (dst - core * SHR).astype(np.int64)
    blk = dl // 128
    pas = blk // BPP
    src_p = (src // SHR) * SH + (src % SHR)
    chunk = src_p // CHUNK
    sloc = (src_p - chunk * CHUNK).astype(np.int16)

    counts = np.zeros((NC, 4, NBLK), dtype=np.int64)
    np.add.at(counts, (core, chunk, blk), 1)
    caps = np.maximum(counts.max(axis=0), 128)          # [4, 98]

    bin_start = np.zeros((4, NBLK), dtype=np.int64)
    run_start = {}
    run_cap = {}
    pos = 0
    for p in range(NPASS):
        for k in range(4):
            run_start[(p, k)] = pos
            for b in _pass_blocks(p):
                bin_start[k, b] = pos
                pos += int(caps[k, b])
            tot = pos - run_start[(p, k)]
            pad = (-tot) % 128
            pos += pad
            run_cap[(p, k)] = tot + pad
    EP = pos
    TP = EP // 128

    b_pri = np.zeros(TP, dtype=np.int64)
    has2 = np.zeros(TP, dtype=bool)
    for p in range(NPASS):
        blocks = list(_pass_blocks(p))
        for k in range(4):
            rs, rc = run_start[(p, k)], run_cap[(p, k)]
            ends = np.cumsum([caps[k, b] for b in blocks])
            for trel in range(rc // 128):
                t = rs // 128 + trel
                p0 = trel * 128
                j = min(int(np.searchsorted(ends, p0, side="right")),
                        len(blocks) - 1)
                b_pri[t] = blocks[j]
                if j + 1 < len(blocks) and ends[j] < p0 + 128:
                    has2[t] = True

    # first/last matmul index per dst block, over run-major emission
    first, last = {}, {}
    i = 0
    for p in range(NPASS):
        for k in range(4):
            rs, rc = run_start[(p, k)], run_cap[(p, k)]
            for trel in range(rc // 128):
                t = rs // 128 + trel
                for b in ([int(b_pri[t])] +
                          ([int(b_pri[t]) + 1] if has2[t] else [])):
                    if b not in first:
                        first[b] = i
                    last[b] = i
                    i += 1

    # position assignment per core
    order_key = np.lexsort((blk, chunk, pas, core))
    ck = (core * 4 + chunk) * NBLK + blk
    ck_sorted = ck[order_key]
    grp_change = np.r_[True, ck_sorted[1:] != ck_sorted[:-1]]
    grp_first = np.where(grp_change)[0]
    grp_id = np.cumsum(grp_change) - 1
    rank = np.arange(E) - grp_first[grp_id]
    bs = bin_start[chunk[order_key], blk[order_key]]
    epos = np.empty(E, dtype=np.int64)
    epos[order_key] = bs + rank

    idx16 = np.zeros((NC, EP), dtype=np.int16)
    slot_a = np.full((NC, EP), -1000.0, dtype=np.float32)
    w_a = np.zeros((NC, EP), dtype=np.float32)
    slot_val = dl - 128 * b_pri[epos // 128]
    assert slot_val.min() >= 0 and slot_val.max() < 256
    idx16[core, epos] = sloc
    slot_a[core, epos] = slot_val
    w_a[core, epos] = w

    idx_l = idx16.reshape(NC, EP // 16, 16).transpose(0, 2, 1)
    idx_rep = np.tile(idx_l, (1, 8, 1)).astype(np.int16)          # [NC,128,EP/16]
    # rep-2 along the tile axis: innermost AP dim becomes a stride-1 pair so
    # the DVE indicator build qualifies for the 2x packed mode.
    slot_t = np.repeat(
        slot_a.reshape(NC, TP, 128).transpose(0, 2, 1), 2, axis=2
    ).astype(ml_dtypes.bfloat16)                                   # [NC,128,2TP]
    w_t = w_a.reshape(NC, TP, 128).transpose(0, 2, 1).astype(ml_dtypes.bfloat16)

    deg = np.zeros(N, dtype=np.float64)
    np.add.at(deg, dst, w.astype(np.float64))
    deg_sh = np.zeros((NC, 1, SH), dtype=np.float32)
    deg_sh[:, 0, :SHR] = deg.astype(np.float32).reshape(NC, SHR)
    deg_rep = np.repeat(deg_sh, D, axis=1).astype(ml_dtypes.bfloat16)
    degone = np.zeros((NC, 2, SH), dtype=np.float32)
    degone[:, 0, :] = deg_sh[:, 0, :]
    degone[:, 1, :SHR] = 1.0
    degone = degone.astype(ml_dtypes.bfloat16)

    xT = np.zeros((NC, 5, SH), dtype=np.float32)
    xT[:, 0:4, :SHR] = x.reshape(NC, SHR, 4).transpose(0, 2, 1)
    xT[:, 4, :SHR] = 1.0
    xTb = xT.astype(ml_dtypes.bfloat16)

    bslot = np.full((NC, SH), -1000.0, dtype=np.float32)
    bslot[:, :SHR] = batch.reshape(NC, SHR)
    bslot_t = np.repeat(
        bslot.reshape(NC, NBLK, 128).transpose(0, 2, 1), 2, axis=2
    ).astype(ml_dtypes.bfloat16)                                   # [NC,128,2NBLK]

    meta = dict(EP=EP, TP=TP, run_start=run_start, run_cap=run_cap,
                b_pri=b_pri, has2=has2, first=first, last=last,
                maxrt=max(rc // 128 for rc in run_cap.values()))
    percore = dict(idxs=idx_rep, slot=slot_t, wt=w_t, degone=degone,
                   deg_rep=deg_rep, xT=xTb, bslot=bslot_t)
    return meta, percore


def prep_weights(W_emb, b_emb, W1, b1, W2, W3, b3, gamma, beta,
                 W_l1, b_l1, W_l2, b_l2):
    bf = ml_dtypes.bfloat16
    f = lambda a: np.asarray(a, np.float32)
    out = {}
    out["Wemb"] = np.concatenate([f(W_emb), f(b_emb)[None, :]], 0).astype(bf)
    out["W1b1"] = np.stack([
        np.concatenate([f(W1[l]), f(b1[l])[None, :], f(b3[l])[None, :]], 0)
        for l in range(L)]).astype(bf)                              # [3,66,64]
    out["W32"] = np.stack([
        np.concatenate([f(W3[l]), -f(W2[l])], 0) for l in range(L)]).astype(bf)
    out["gamT"] = f(gamma).reshape(L, D, 1)
    out["betT"] = f(beta).reshape(L, D, 1)
    out["Wl1"] = np.concatenate([f(W_l1), f(b_l1)[None, :]], 0).astype(bf)
    out["Wl2"] = np.concatenate([f(W_l2), f(b_l2)[None, :]], 0).astype(bf)
    out["identb"] = np.eye(128, dtype=np.float32).astype(bf)
    out["identf"] = np.eye(128, dtype=np.float32)
    out["iota"] = np.tile(np.arange(256, dtype=np.float32)[None, :],
                          (128, 1)).astype(bf)
    return out


# ----------------------------------------------------------------------------
# Device graph
# ----------------------------------------------------------------------------

def build(meta, debug=False):
    EP, TP = meta["EP"], meta["TP"]
    run_start, run_cap = meta["run_start"], meta["run_cap"]
    b_pri, has2 = meta["b_pri"], meta["has2"]
    first, last = meta["first"], meta["last"]
    MAXRT = meta["maxrt"]

    nc = bacc.Bacc("TRN2", debug=False, num_swdge_queues=4)

    d_idxs = nc.dram_tensor("idxs", [128, EP // 16], I16, kind="ExternalInput")
    d_slot = nc.dram_tensor("slot", [128, 2 * TP], BF16, kind="ExternalInput")
    d_wt = nc.dram_tensor("wt", [128, TP], BF16, kind="ExternalInput")
    d_deg = nc.dram_tensor("degone", [2, SH], BF16, kind="ExternalInput")
    d_degr = nc.dram_tensor("deg_rep", [D, SH], BF16, kind="ExternalInput")
    d_xT = nc.dram_tensor("xT", [5, SH], BF16, kind="ExternalInput")
    d_bslot = nc.dram_tensor("bslot", [128, 2 * NBLK], BF16, kind="ExternalInput")
    d_Wemb = nc.dram_tensor("Wemb", [5, D], BF16, kind="ExternalInput")
    d_W1b1 = nc.dram_tensor("W1b1", [L, 66, D], BF16, kind="ExternalInput")
    d_W32 = nc.dram_tensor("W32", [L, 128, D], BF16, kind="ExternalInput")
    d_gamT = nc.dram_tensor("gamT", [L, D, 1], F32, kind="ExternalInput")
    d_betT = nc.dram_tensor("betT", [L, D, 1], F32, kind="ExternalInput")
    d_Wl1 = nc.dram_tensor("Wl1", [65, D], BF16, kind="ExternalInput")
    d_Wl2 = nc.dram_tensor("Wl2", [65, 3], BF16, kind="ExternalInput")
    d_identb = nc.dram_tensor("identb", [128, 128], BF16, kind="ExternalInput")
    d_identf = nc.dram_tensor("identf", [128, 128], F32, kind="ExternalInput")
    d_iota = nc.dram_tensor("iota", [128, 256], BF16, kind="ExternalInput")
    d_out = nc.dram_tensor("out", [G, 3], F32, kind="ExternalOutput")
    d_tbls = [nc.dram_tensor(f"tbl{i}", [NP, 128], BF16, addr_space="Shared")
              for i in range(L)]
    d_dbg = (nc.dram_tensor("dbg", [5, 128, SH], BF16, kind="ExternalOutput")
             if debug else None)
    d_tdump = (nc.dram_tensor("tdump", [NP, 128], BF16, kind="ExternalOutput")
               if debug else None)
    d_gdump = (nc.dram_tensor("gdump", [128, meta["maxrt"], 128], BF16,
                              kind="ExternalOutput") if debug else None)

    AluOp = mybir.AluOpType
    AF = mybir.ActivationFunctionType

    with tile.TileContext(nc) as tc:
        with (
            tc.tile_pool(name="const", bufs=1) as cp,
            tc.tile_pool(name="big", bufs=1) as bigp,
            tc.tile_pool(name="gath", bufs=8) as gp,
            tc.tile_pool(name="ind", bufs=3) as ip,
            tc.tile_pool(name="ixp", bufs=8) as ixp,
            tc.tile_pool(name="work", bufs=1) as wp,
            tc.tile_pool(name="dram", bufs=1, space="DRAM") as dp,
            tc.tile_pool(name="dram2", bufs=2, space="DRAM") as dp2,
        ):
            from contextlib import ExitStack
            ps_stack = ExitStack()
            psP = ps_stack.enter_context(
                tc.tile_pool(name="psP", bufs=4, space="PSUM"))
            psE = ps_stack.enter_context(
                tc.tile_pool(name="psE", bufs=2, space="PSUM"))
            psT = ps_stack.enter_context(
                tc.tile_pool(name="psT", bufs=2, space="PSUM"))
            nc.gpsimd.load_library(mlp_lib)

            def load(pool, dten, tag):
                t = pool.tile(list(dten.shape), dten.dtype, tag=tag)
                nc.sync.dma_start(t[:], dten[:])
                return t

            slot = load(cp, d_slot, "slot")
            wt = load(cp, d_wt, "wt")
            degr = load(cp, d_degr, "degr")
            xT = load(cp, d_xT, "xT")
            bslot = load(cp, d_bslot, "bslot")
            Wemb = load(cp, d_Wemb, "Wemb")
            gamT = cp.tile([D, L], F32, tag="gamT")
            nc.sync.dma_start(gamT[:], d_gamT[:].rearrange("l p one -> p (l one)"))
            betT = cp.tile([D, L], F32, tag="betT")
            nc.sync.dma_start(betT[:], d_betT[:].rearrange("l p one -> p (l one)"))
            Wl1 = load(cp, d_Wl1, "Wl1")
            Wl2 = load(cp, d_Wl2, "Wl2")
            identb = load(cp, d_identb, "identb")
            identf = load(cp, d_identf, "identf")
            iota = load(cp, d_iota, "iota")
            W1b1 = cp.tile([66, L, D], BF16, tag="W1b1")
            nc.sync.dma_start(W1b1[:], d_W1b1[:].rearrange("l p f -> p l f"))
            W32 = cp.tile([128, L, D], BF16, tag="W32")
            nc.sync.dma_start(W32[:], d_W32[:].rearrange("l p f -> p l f"))

            hcur = bigp.tile([128, SH], BF16)      # 0:64 hT, 64:128 deg.hT
            hpre = bigp.tile([D, SH], BF16)        # pre-BN h'
            pT = bigp.tile([66, SH], BF16)         # 0:64 p^T, 64 deg, 65 ones
            hnode = bigp.tile([128, NBLK * 128], BF16)

            nc.sync.dma_start(pT[64:66, :], d_deg[:])

            def dense_chunks():
                o = 0
                while o < SH:
                    yield o, min(512, SH - o)
                    o += 512

            def deg_mult():
                nc.vector.tensor_tensor(hcur[D:128, :], hcur[0:D, :], degr[:],
                                        op=AluOp.mult)

            def emb_layer():
                for o, n in dense_chunks():
                    acc = psE.tile([D, 512], F32, tag="acc")
                    nc.tensor.matmul(acc[:, 0:n], Wemb[:], xT[:, o:o + n],
                                     start=True, stop=True)
                    nc.scalar.activation(hcur[0:D, o:o + n], acc[:, 0:n], AF.Copy)
                deg_mult()

            def to_node_major():
                for b in range(NBLK):
                    tp = psT.tile([128, 64], BF16, tag="tp")
                    nc.tensor.transpose(tp[:], hcur[0:D, 128 * b:128 * (b + 1)],
                                        identb[0:D, 0:D])
                    nc.vector.tensor_copy(hnode[:, b * 128:b * 128 + 64], tp[:])

            def allgather_table(i):
                bounce = dp2.tile([SH, 128], BF16, tag="agin")
                tbl = d_tbls[i]
                nc.sync.dma_start(
                    bounce[:].rearrange("(t p) e -> p t e", p=128),
                    hnode[:].rearrange("p (t e) -> p t e", e=128))
                nc.gpsimd.collective_compute(
                    "AllGather", AluOp.bypass,
                    replica_groups=[list(range(NC))],
                    ins=[bounce.opt()], outs=[tbl[:].opt()])
                return tbl

            def spmm(tbl, dumpg=False):
                mmc = 0
                for p in range(NPASS):
                    blocks = list(_pass_blocks(p))
                    paccs = {b: psP.tile([D, 128], F32, tag="pacc",
                                         name=f"pacc{b}")
                             for b in blocks}
                    gts = {}
                    for k in range(4):
                        rs, rc = run_start[(p, k)], run_cap[(p, k)]
                        gt = gp.tile([128, MAXRT, 128], BF16, tag="gt")
                        idxt = ixp.tile([128, MAXRT * 8], I16, tag="idxt")
                        nc.sync.dma_start(
                            idxt[:, 0:rc // 16],
                            d_idxs[:, rs // 16:(rs + rc) // 16])
                        off = 0
                        while off < rc:
                            n = min(MAXCALL, rc - off)
                            nc.gpsimd.dma_gather(
                                gt[:, off // 128:(off + n) // 128, :],
                                tbl[k * CHUNK:(k + 1) * CHUNK, :],
                                idxt[:, off // 16:(off + n) // 16],
                                n, n, 128, single_packet=False, queue_num=k)
                            off += n
                        gts[k] = gt
                    for k in range(4):
                        rs, rc = run_start[(p, k)], run_cap[(p, k)]
                        nt = rc // 128
                        t_base = rs // 128
                        gt = gts[k]
                        # fold w into the gathered rows (per-edge scale)
                        nc.vector.tensor_tensor(
                            gt[:, 0:nt, 0:64], gt[:, 0:nt, 0:64],
                            wt[:, t_base:t_base + nt].unsqueeze(-1)
                                .broadcast_to((128, nt, 64)),
                            op=AluOp.mult)
                        if dumpg and p == 0 and k == 0:
                            nc.sync.dma_start(d_gdump[:], gt[:])
                        for t0 in range(0, nt, TB):
                            tbn = min(TB, nt - t0)
                            tg0 = t_base + t0
                            ind = ip.tile([128, TB, 256], BF16, tag="ind")
                            nc.vector.tensor_tensor(
                                ind[:, 0:tbn, :].rearrange(
                                    "p t (o i) -> p t o i", i=2),
                                iota[:].rearrange("p (o i) -> p o i", i=2)
                                    .unsqueeze(1)
                                    .broadcast_to((128, tbn, 128, 2)),
                                slot[:, 2 * tg0:2 * (tg0 + tbn)].rearrange(
                                    "p (t i) -> p t i", i=2)
                                    .unsqueeze(2)
                                    .broadcast_to((128, tbn, 128, 2)),
                                op=AluOp.is_equal)
                            for trel in range(tbn):
                                tg = tg0 + trel
                                for half in (0, 1):
                                    if half == 1 and not has2[tg]:
                                        continue
                                    b = int(b_pri[tg]) + half
                                    nc.tensor.matmul(
                                        paccs[b][:],
                                        gt[:, t0 + trel, 0:64],
                                        ind[:, trel, half * 128:half * 128 + 128],
                                        start=(first[b] == mmc),
                                        stop=(last[b] == mmc),
                                        skip_group_check=True)
                                    mmc += 1
                    for b in blocks:
                        nc.scalar.activation(pT[0:D, b * 128:(b + 1) * 128],
                                             paccs[b][:], AF.Copy)

            def epilogue(lidx):
                for o, n in dense_chunks():
                    acc = psE.tile([D, 512], F32, tag="acc")
                    nc.tensor.matmul(acc[:, 0:n], W1b1[:, lidx, :], pT[:, o:o + n],
                                     start=True, stop=False)
                    nc.tensor.matmul(acc[:, 0:n], W32[:, lidx, :], hcur[:, o:o + n],
                                     start=False, stop=True)
                    nc.scalar.activation(hpre[:, o:o + n], acc[:, 0:n], AF.Copy)
                nc.vector.memset(hpre[:, SHR:SH], 0.0)

            def bn_relu(lidx):
                s = wp.tile([D, 2], F32, tag="stats")
                nc.vector.tensor_reduce(s[:, 0:1], hpre[:], mybir.AxisListType.X,
                                        AluOp.add)
                nc.scalar.square(pT[0:D, :], hpre[:])
                nc.vector.tensor_reduce(s[:, 1:2], pT[0:D, :],
                                        mybir.AxisListType.X, AluOp.add)
                bnc = dp.tile([D, 2], F32, tag="bnin")
                bno = dp.tile([D, 2], F32, tag="bnout", addr_space="Shared")
                nc.sync.dma_start(bnc[:], s[:])
                nc.gpsimd.collective_compute(
                    "AllReduce", AluOp.add, replica_groups=[list(range(NC))],
                    ins=[bnc.opt()], outs=[bno.opt()])
                sg = wp.tile([D, 2], F32, tag="statsg")
                nc.sync.dma_start(sg[:], bno[:])
                mean = wp.tile([D, 1], F32, tag="mean")
                var = wp.tile([D, 1], F32, tag="var")
                alph = wp.tile([D, 1], F32, tag="alph")
                bet2 = wp.tile([D, 1], F32, tag="bet2")
                nc.vector.tensor_scalar_mul(mean[:], sg[:, 0:1], 1.0 / N)
                nc.vector.tensor_scalar_mul(var[:], sg[:, 1:2], 1.0 / N)
                nc.vector.tensor_tensor(alph[:], mean[:], mean[:], op=AluOp.mult)
                nc.vector.tensor_tensor(var[:], var[:], alph[:], op=AluOp.subtract)
                nc.vector.tensor_scalar_add(var[:], var[:], BN_EPS)
                nc.scalar.sqrt(var[:], var[:])
                nc.vector.reciprocal(var[:], var[:])
                nc.vector.tensor_tensor(alph[:], gamT[:, lidx:lidx+1], var[:],
                                        op=AluOp.mult)
                nc.vector.tensor_tensor(bet2[:], mean[:], alph[:], op=AluOp.mult)
                nc.vector.tensor_tensor(bet2[:], betT[:, lidx:lidx+1], bet2[:],
                                        op=AluOp.subtract)
                nc.scalar.activation(hcur[0:D, :], hpre[:], AF.Relu,
                                     bias=bet2[:], scale=alph[:])
                nc.vector.memset(hcur[0:D, SHR:SH], 0.0)
                deg_mult()

            # ---------------- main program ----------------
            def dump(i):
                if d_dbg is not None:
                    nc.sync.dma_start(d_dbg[i], hcur[:])
            emb_layer()
            dump(0)
            to_node_major()
            tbl = allgather_table(0)
            if d_tdump is not None:
                nc.sync.dma_start(d_tdump[:], tbl[:])
            for lidx in range(L):
                spmm(tbl, dumpg=(d_gdump is not None and lidx == 0))
                if d_dbg is not None and lidx == 0:
                    nc.sync.dma_start(d_dbg[4, 0:66, :], pT[:])
                epilogue(lidx)
                bn_relu(lidx)
                dump(1 + lidx)
                if lidx < L - 1:
                    to_node_major()
                    tbl = allgather_table(lidx + 1)

            # ---------------- pooling + MLP head ----------------
            to_node_major()
            ps_stack.close()
            psG = ps_stack.enter_context(
                tc.tile_pool(name="psG", bufs=2, space="PSUM"))
            psM = ps_stack.enter_context(
                tc.tile_pool(name="psM", bufs=2, space="PSUM"))
            hv3 = hnode[:].rearrange("p (t e) -> p t e", e=128)
            nc.vector.memset(hv3[:, :, 64:65], 1.0)
            gacc0 = psG.tile([128, 65], F32, tag="gacc")
            gacc1 = psG.tile([128, 65], F32, tag="gacc")
            for t0 in range(0, NBLK, TB):
                tbn = min(TB, NBLK - t0)
                pind = ip.tile([128, TB, 256], BF16, tag="ind")
                nc.vector.tensor_tensor(
                    pind[:, 0:tbn, :].rearrange("p t (o i) -> p t o i", i=2),
                    iota[:].rearrange("p (o i) -> p o i", i=2)
                        .unsqueeze(1).broadcast_to((128, tbn, 128, 2)),
                    bslot[:, 2 * t0:2 * (t0 + tbn)].rearrange(
                        "p (t i) -> p t i", i=2)
                        .unsqueeze(2).broadcast_to((128, tbn, 128, 2)),
                    op=AluOp.is_equal)
                for trel in range(tbn):
                    t = t0 + trel
                    nc.tensor.matmul(gacc0[:], pind[:, trel, 0:128],
                                     hv3[:, t, 0:65],
                                     start=(t == 0), stop=(t == NBLK - 1),
                                     skip_group_check=True)
                    nc.tensor.matmul(gacc1[:], pind[:, trel, 128:256],
                                     hv3[:, t, 0:65],
                                     start=(t == 0), stop=(t == NBLK - 1),
                                     skip_group_check=True)
            gsb = wp.tile([128, 2, 65], F32, tag="gsb")
            nc.vector.tensor_copy(gsb[:, 0, :], gacc0[:])
            nc.vector.tensor_copy(gsb[:, 1, :], gacc1[:])
            pc_in = dp.tile([128, 2, 65], F32, tag="plin")
            pc_out = dp.tile([128, 2, 65], F32, tag="plout", addr_space="Shared")
            nc.sync.dma_start(pc_in[:], gsb[:])
            nc.gpsimd.collective_compute(
                "AllReduce", AluOp.add, replica_groups=[list(range(NC))],
                ins=[pc_in.opt()], outs=[pc_out.opt()])
            nc.sync.dma_start(gsb[:], pc_out[:])
            gT = wp.tile([65, 256], BF16, tag="gT")
            for wdw in range(2):
                cnt = wp.tile([128, 1], F32, tag="cnt")
                nc.vector.tensor_scalar_max(cnt[:], gsb[:, wdw, 64:65], 1.0)
                nc.vector.reciprocal(cnt[:], cnt[:])
                nc.vector.tensor_scalar(gsb[:, wdw, 0:64], gsb[:, wdw, 0:64],
                                        cnt[:], None, op0=AluOp.mult)
                tp2 = psM.tile([128, 256], F32, tag="mlp")
                nc.tensor.transpose(tp2[0:64, 0:128], gsb[:, wdw, 0:64], identf[:])
                nc.vector.tensor_copy(gT[0:64, wdw * 128:(wdw + 1) * 128], tp2[0:64, 0:128])
            nc.vector.memset(gT[64:65, :], 1.0)
            z = psM.tile([128, 256], F32, tag="mlp")
            nc.tensor.matmul(z[0:64, :], Wl1[:], gT[:], start=True, stop=True)
            zsb = wp.tile([65, 256], BF16, tag="zsb")
            nc.scalar.activation(zsb[0:64, :], z[0:64, :], AF.Relu)
            nc.vector.memset(zsb[64:65, :], 1.0)
            pred = psM.tile([128, 256], F32, tag="mlp")
            nc.tensor.matmul(pred[0:3, :], Wl2[:], zsb[:], start=True, stop=True)
            psb = wp.tile([3, 256], F32, tag="psb")
            nc.vector.tensor_copy(psb[:], pred[0:3, :])
            for hf in range(2):
                tp3 = psM.tile([128, 256], F32, tag="mlp")
                nc.tensor.transpose(tp3[0:128, 0:3], psb[:, hf * 128:(hf + 1) * 128],
                                    identf[0:3, 0:3])
                osb = wp.tile([128, 3], F32, tag="osb")
                nc.vector.tensor_copy(osb[:], tp3[0:128, 0:3])
                nc.sync.dma_start(d_out[hf * 128:(hf + 1) * 128, :], osb[:])
            ps_stack.close()

    nc.compile()
    return nc


# ----------------------------------------------------------------------------
# Entry point
# ----------------------------------------------------------------------------

_cache = {}


def kernel(x, EdgeID, EdgeAttr, batch, W_emb, b_emb, W1, b1, W2, W3, b3,
           gamma, beta, W_l1, b_l1, W_l2, b_l2, _trace=False, _debug=False):
    meta, percore = preprocess(x, EdgeID, EdgeAttr, batch)
    wts = prep_weights(W_emb, b_emb, W1, b1, W2, W3, b3, gamma, beta,
                       W_l1, b_l1, W_l2, b_l2)
    key = ("nc", meta["EP"], bool(_debug))
    if key not in _cache:
        _cache[key] = build(meta, debug=_debug)
    nc = _cache[key]
    in_maps = []
    for c in range(NC):
        m = dict(wts)
        m["idxs"] = percore["idxs"][c]
        m["slot"] = percore["slot"][c]
        m["wt"] = percore["wt"][c]
        m["degone"] = percore["degone"][c]
        m["deg_rep"] = percore["deg_rep"][c]
        m["xT"] = percore["xT"][c]
        m["bslot"] = percore["bslot"][c]
        in_maps.append(m)
    res = run_bass_kernel_spmd(nc, in_maps, list(range(NC)), trace=_trace)
    out = np.asarray(res.results[0]["out"], dtype=np.float32)
    kernel.last_result = res
    if _trace:
        kernel.last_exec_ns = res.exec_time_ns
    if _debug:
        kernel.dbg = [np.asarray(r["dbg"]) for r in res.results]
        kernel.tdump = [np.asarray(r["tdump"]) for r in res.results]
        kernel.gdump = [np.asarray(r["gdump"]) for r in res.results]
    return out

